# revision 19
# baseline (speedup 1.0000x reference)
"""Trainium2 Bass kernel for nn_BackwardTransformLayer (inverse wavelet step).

Math (polyphase form of the reference):
    g = flip(scaling_rec); g[1::2] *= -1
    out[i, 2u+p] = sum_{j=0..3} g[2j+p] * d[i, (u+j+p) % M]
                 + sum_{j=0..3} s[2j+p] * a[i, (u+j+p) % M]      (p in {0,1})

Formulated as banded matmuls on the PE with ZERO input duplication: input
tile t packs d^T rows [64t, 64t+64) on partitions 0..63 and a^T rows on
64..127 (non-overlapping -> each input byte is DMA'd exactly once).  Block b
produces output columns [128b, 128b+128) as a 2-matmul PSUM accumulation:

    psum  = W_main.T @ tile_b          (taps that fall inside tile b)
          + W_halo.T @ tile_{b+1}      (the 4-row spill into the next tile)

W_main/W_halo [128, 128] fp16 are shift-invariant (same for every block);
the last block's spill uses an 8-partition tail tile (d/a halo rows) with a
compacted W_tail [8, 128].  Matmul cost is N cycles regardless of K, so the
halo matmul is free; PSUM accumulates in fp32.

The kernel is purely HBM-bandwidth-bound, so both sides are 16-bit on the
wire: inputs fp16, and the fp32 PSUM result is rounded to fp16 during the
PSUM->SBUF drain and widened back to fp32 on the host (~5e-4 rel error,
gate is 2e-2).

Sharding: by u-columns (embarrassingly parallel, 4-col halo replicated
host-side): core c handles u in [1024c, 1024(c+1)), all 4096 rows.  Per
core and rep: 16 full-tile in-DMAs (1 MB) + one 64 KB tail DMA on the sync
ring, 256 matmuls (N=512), 128 PSUM drains (ACT/DVE alternating, casting
to fp16), 16 out-DMAs ([128, 4096] fp16, ACT ring).  Output is computed
transposed ([2048, 4096] per core) and transposed back on the host.
"""

import numpy as np

P = 128                      # SBUF partitions
M = 8192                     # input columns (u dimension)
N_ROWS = 4096
N_CORES = 8
FLEN = 8
U_PER_CORE = M // N_CORES    # 1024
TU = 64                      # u-columns per tile and per block
NBLK = U_PER_CORE // TU      # 16
X_ROWS = NBLK * P + 8        # 2056: 16 full tiles + 8-row tail tile
OUT_W = 2 * U_PER_CORE       # 2048 output columns per core
NCHUNK = N_ROWS // 512       # 8 moving-dim chunks of 512 rows
WCOLS = 3 * P                # [main | halo | tail(rows 0..7)]

_CACHE = {}


def _build(reps=1, staggered=False, in_rings=("sync",)):
    import contextlib

    import concourse.bacc as bacc
    import concourse.mybir as mybir
    from concourse.tile import TileContext

    f32 = mybir.dt.float32
    f16 = mybir.dt.float16

    nc = bacc.Bacc("TRN2", target_bir_lowering=False, debug=False)
    x = nc.dram_tensor("x", [X_ROWS, N_ROWS], f16, kind="ExternalInput")
    w = nc.dram_tensor("w", [P, WCOLS], f16, kind="ExternalInput")
    o = nc.dram_tensor("o", [OUT_W, N_ROWS], f16, kind="ExternalOutput")

    with TileContext(nc) as tc:
        with (
            tc.tile_pool(name="const", bufs=1) as const_pool,
            tc.tile_pool(name="xin", bufs=6) as xin_pool,
            tc.tile_pool(name="out", bufs=3) as out_pool,
            tc.tile_pool(name="psum", bufs=8, space="PSUM") as psum_pool,
        ):
            w_sb = const_pool.tile([P, WCOLS], f16)
            nc.sync.dma_start(out=w_sb[:], in_=w[:])

            rep_ctx = (
                tc.For_i(0, reps, 1, staggered_reset=staggered)
                if reps > 1
                else contextlib.nullcontext()
            )
            with rep_ctx:
                def load_tile(t):
                    tt = xin_pool.tile([P, N_ROWS], f16, tag="x")
                    rows = P if t < NBLK else 8
                    in_eng = getattr(nc, in_rings[t % len(in_rings)])
                    in_eng.dma_start(
                        out=tt[:rows, :], in_=x[t * P:t * P + rows, :]
                    )
                    return tt

                cur = load_tile(0)
                for b in range(NBLK):
                    nxt = load_tile(b + 1)
                    o_t = out_pool.tile([P, N_ROWS], f16, tag="o")
                    r0 = P * b
                    for ch in range(NCHUNK):
                        n0 = 512 * ch
                        ps = psum_pool.tile([P, 512], f32, tag="ps")
                        nc.tensor.matmul(
                            ps[:],
                            w_sb[:, 0:P],
                            cur[:, n0:n0 + 512],
                            start=True,
                            stop=False,
                        )
                        if b < NBLK - 1:
                            nc.tensor.matmul(
                                ps[:],
                                w_sb[:, P:2 * P],
                                nxt[:, n0:n0 + 512],
                                start=False,
                                stop=True,
                            )
                        else:
                            nc.tensor.matmul(
                                ps[:],
                                w_sb[0:8, 2 * P:3 * P],
                                nxt[0:8, n0:n0 + 512],
                                start=False,
                                stop=True,
                            )
                        if ch % 2 == 0:
                            nc.scalar.copy(out=o_t[:, n0:n0 + 512], in_=ps[:])
                        else:
                            nc.vector.tensor_copy(
                                out=o_t[:, n0:n0 + 512], in_=ps[:]
                            )
                    nc.scalar.dma_start(out=o[r0:r0 + P, :], in_=o_t[:])
                    cur = nxt
    nc.compile()
    return nc


def _make_w(scaling, scaling_rec):
    s = np.asarray(scaling, dtype=np.float64)
    sr = np.asarray(scaling_rec, dtype=np.float64)
    g = sr[::-1].copy()
    g[1::2] *= -1.0

    wm = np.zeros((P, P), np.float64)   # taps inside tile b
    wh = np.zeros((P, P), np.float64)   # 4-row spill into tile b+1
    for ul in range(TU):
        for p in range(2):
            m = 2 * ul + p
            for j in range(4):
                k = ul + j + p
                if k < TU:
                    wm[k, m] = g[2 * j + p]
                    wm[TU + k, m] = s[2 * j + p]
                else:
                    wh[k - TU, m] = g[2 * j + p]
                    wh[TU + (k - TU), m] = s[2 * j + p]
    wt = np.zeros((P, P), np.float64)   # compacted spill for the tail tile
    wt[0:4] = wh[0:4]                   # d halo rows
    wt[4:8] = wh[TU:TU + 4]             # a halo rows
    return np.concatenate([wm, wh, wt], axis=1).astype(np.float16)


def make_in_maps(details, approximation, scaling, scaling_rec):
    d16 = np.asarray(details, dtype=np.float16)
    a16 = np.asarray(approximation, dtype=np.float16)
    # circular halo: the tail tile reaches cols [1024c + 1024, 1024c + 1028)
    dT = np.ascontiguousarray(
        np.concatenate([d16, d16[:, :FLEN]], axis=1).T
    )  # [M+8, N_ROWS]
    aT = np.ascontiguousarray(np.concatenate([a16, a16[:, :FLEN]], axis=1).T)
    w_np = _make_w(scaling, scaling_rec)

    in_maps = []
    for c in range(N_CORES):
        xc = np.zeros((X_ROWS, N_ROWS), np.float16)
        u0 = U_PER_CORE * c
        for t in range(NBLK):
            xc[t * P:t * P + TU] = dT[u0 + TU * t:u0 + TU * (t + 1)]
            xc[t * P + TU:(t + 1) * P] = aT[u0 + TU * t:u0 + TU * (t + 1)]
        bt = NBLK * P
        xc[bt:bt + 4] = dT[u0 + U_PER_CORE:u0 + U_PER_CORE + 4]
        xc[bt + 4:bt + 8] = aT[u0 + U_PER_CORE:u0 + U_PER_CORE + 4]
        in_maps.append({"x": xc, "w": w_np})
    return in_maps


def kernel(details, approximation, scaling, scaling_rec):
    if "nc" not in _CACHE:
        _CACHE["nc"] = _build()
    nc = _CACHE["nc"]

    from concourse.bass_utils import run_bass_kernel_spmd

    in_maps = make_in_maps(details, approximation, scaling, scaling_rec)
    res = run_bass_kernel_spmd(nc, in_maps, core_ids=list(range(N_CORES)))
    out = np.empty((N_ROWS, 2 * M), np.float32)
    for c in range(N_CORES):
        out[:, OUT_W * c:OUT_W * (c + 1)] = (
            res.results[c]["o"].astype(np.float32).T
        )
    return out


# revision 21
# speedup vs baseline: 1.0738x; 1.0738x over previous
"""Trainium2 Bass kernel for nn_BackwardTransformLayer (inverse wavelet step).

Math (polyphase form of the reference):
    g = flip(scaling_rec); g[1::2] *= -1
    out[i, 2u]   = sum_{j=0..3} g[2j]   * d[i, (u+j)   % M] + s[2j]   * a[i, (u+j)   % M]
    out[i, 2u+1] = sum_{j=0..3} g[2j+1] * d[i, (u+1+j) % M] + s[2j+1] * a[i, (u+1+j) % M]

Formulated as ONE banded matmul on the PE: for a block of BU=60 consecutive
u-columns, the 2*BU output columns are W.T @ X where
  - X [128, nrows]: partitions 0..63   = d^T rows [u0, u0+64)   (BU + 4 halo)
                    partitions 64..127 = a^T rows [u0, u0+64)
  - W [128, 2*BU] fp16: the shift-invariant banded coefficient matrix
    (4 taps per output column from each source), identical for every block.
PSUM accumulates in fp32; matmul cost is N cycles regardless of K, so this
does all 16 MACs per output element in 1/8 matmul-column-cycle each.

The kernel is purely HBM-bandwidth-bound, so both sides are 16-bit on the
wire: inputs fp16, and the fp32 PSUM result is rounded to fp16 during the
PSUM->SBUF drain and widened back to fp32 on the host (~5e-4 rel error,
gate is 2e-2).  The tail block (4 u-columns) gets a dedicated 16-partition
W slice (cols 2*BU..2*BU+2*TAIL_U) so its input tile is [16, nrows].

Sharding: by u-columns (embarrassingly parallel, 4-col halo replicated
host-side): core c handles u in [1024c, 1024(c+1)), all 4096 rows.  Inputs
are packed host-side into per-block [128, 4096] fp16 tiles (1 MB DMAs, all
128 partitions); output is computed transposed ([2048, 4096] per core) and
transposed back on the host.

Per core: 17 full blocks (60 u) + 1 tail block (4 u): 18 in-DMAs (sync
ring), 144 matmuls (N=512), 144 PSUM drains (ACT/DVE alternating, casting
to fp16), 18 out-DMAs (ACT ring).
"""

import numpy as np

P = 128                      # SBUF partitions
M = 8192                     # input columns (u dimension)
N_ROWS = 4096
N_CORES = 8
FLEN = 8
U_PER_CORE = M // N_CORES    # 1024
BU = 60                      # u-columns per full block (BU + 4 halo = 64 per source)
NBLK_FULL = 17               # 17 * 60 = 1020
TAIL_U = U_PER_CORE - NBLK_FULL * BU   # 4
TAIL_K = 2 * (TAIL_U + 4)    # 16 input partitions for the tail block
NBLK = NBLK_FULL + 1         # 18
OUT_W = 2 * U_PER_CORE       # 2048 output columns per core
NCHUNK = N_ROWS // 512       # 8 moving-dim chunks of 512 rows
X_ROWS = NBLK_FULL * P + TAIL_K        # 2192
WCOLS = 2 * BU + 2 * TAIL_U  # 128: full-block W | tail W

_CACHE = {}


def _build(reps=1, staggered=False, in_rings=("sync",)):
    import contextlib

    import concourse.bacc as bacc
    import concourse.mybir as mybir
    from concourse.tile import TileContext

    f32 = mybir.dt.float32
    f16 = mybir.dt.float16

    nc = bacc.Bacc("TRN2", target_bir_lowering=False, debug=False)
    x = nc.dram_tensor("x", [X_ROWS, N_ROWS], f16, kind="ExternalInput")
    w = nc.dram_tensor("w", [P, WCOLS], f16, kind="ExternalInput")
    o = nc.dram_tensor("o", [OUT_W, N_ROWS], f16, kind="ExternalOutput")

    with TileContext(nc) as tc:
        with (
            tc.tile_pool(name="const", bufs=1) as const_pool,
            tc.tile_pool(name="xin", bufs=4) as xin_pool,
            tc.tile_pool(name="out", bufs=3) as out_pool,
            tc.tile_pool(name="psum", bufs=8, space="PSUM") as psum_pool,
        ):
            w_sb = const_pool.tile([P, WCOLS], f16)
            nc.sync.dma_start(out=w_sb[:], in_=w[:])

            rep_ctx = (
                tc.For_i(0, reps, 1, staggered_reset=staggered)
                if reps > 1
                else contextlib.nullcontext()
            )
            with rep_ctx:
                for b in [NBLK - 1] + list(range(NBLK - 1)):
                    x_t = xin_pool.tile([P, N_ROWS], f16, tag="x")
                    in_eng = getattr(nc, in_rings[b % len(in_rings)])
                    if b < NBLK_FULL:
                        bw = 2 * BU
                        w_ap = w_sb[:, :bw]
                        in_eng.dma_start(
                            out=x_t[:], in_=x[b * P:(b + 1) * P, :]
                        )
                    else:
                        bw = 2 * TAIL_U
                        w_ap = w_sb[:TAIL_K, 2 * BU:2 * BU + bw]
                        in_eng.dma_start(
                            out=x_t[:TAIL_K, :],
                            in_=x[b * P:b * P + TAIL_K, :],
                        )
                    o_t = out_pool.tile([2 * BU, N_ROWS], f16, tag="o")
                    r0 = 2 * BU * b
                    for ch in range(NCHUNK):
                        n0 = 512 * ch
                        ps = psum_pool.tile([2 * BU, 512], f32, tag="ps")
                        nc.tensor.matmul(
                            ps[:bw, :],
                            w_ap,
                            x_t[:w_ap.partition_size(), n0:n0 + 512],
                            start=True,
                            stop=True,
                        )
                        if ch % 2 == 0:
                            nc.scalar.copy(out=o_t[:bw, n0:n0 + 512], in_=ps[:bw, :])
                        else:
                            nc.vector.tensor_copy(
                                out=o_t[:bw, n0:n0 + 512], in_=ps[:bw, :]
                            )
                    nc.scalar.dma_start(out=o[r0:r0 + bw, :], in_=o_t[:bw, :])
    nc.compile()
    return nc


def _make_w(scaling, scaling_rec):
    s = np.asarray(scaling, dtype=np.float64)
    sr = np.asarray(scaling_rec, dtype=np.float64)
    g = sr[::-1].copy()
    g[1::2] *= -1.0

    w = np.zeros((P, WCOLS), np.float64)
    for ul in range(BU):
        for j in range(4):
            w[ul + j, 2 * ul] = g[2 * j]
            w[ul + 1 + j, 2 * ul + 1] = g[2 * j + 1]
            w[64 + ul + j, 2 * ul] = s[2 * j]
            w[64 + ul + 1 + j, 2 * ul + 1] = s[2 * j + 1]
    # tail block: d rows on partitions 0..TAIL_U+3, a rows on the next 8
    half = TAIL_U + 4
    for ul in range(TAIL_U):
        for j in range(4):
            w[ul + j, 2 * BU + 2 * ul] = g[2 * j]
            w[ul + 1 + j, 2 * BU + 2 * ul + 1] = g[2 * j + 1]
            w[half + ul + j, 2 * BU + 2 * ul] = s[2 * j]
            w[half + ul + 1 + j, 2 * BU + 2 * ul + 1] = s[2 * j + 1]
    return w.astype(np.float16)


def make_in_maps(details, approximation, scaling, scaling_rec):
    d16 = np.asarray(details, dtype=np.float16)
    a16 = np.asarray(approximation, dtype=np.float16)
    # circular halo: block slices reach at most col 1024c + 1020 + 8 <= M + 8
    dT = np.ascontiguousarray(
        np.concatenate([d16, d16[:, :FLEN]], axis=1).T
    )  # [M+8, N_ROWS]
    aT = np.ascontiguousarray(np.concatenate([a16, a16[:, :FLEN]], axis=1).T)
    w_np = _make_w(scaling, scaling_rec)

    in_maps = []
    for c in range(N_CORES):
        xc = np.zeros((X_ROWS, N_ROWS), np.float16)
        for b in range(NBLK_FULL):
            u0 = U_PER_CORE * c + BU * b
            xc[b * P:b * P + 64] = dT[u0:u0 + 64]
            xc[b * P + 64:b * P + 128] = aT[u0:u0 + 64]
        u0t = U_PER_CORE * c + BU * NBLK_FULL
        bt = NBLK_FULL * P
        half = TAIL_U + 4
        xc[bt:bt + half] = dT[u0t:u0t + half]
        xc[bt + half:bt + 2 * half] = aT[u0t:u0t + half]
        in_maps.append({"x": xc, "w": w_np})
    return in_maps


def kernel(details, approximation, scaling, scaling_rec):
    if "nc" not in _CACHE:
        _CACHE["nc"] = _build()
    nc = _CACHE["nc"]

    from concourse.bass_utils import run_bass_kernel_spmd

    in_maps = make_in_maps(details, approximation, scaling, scaling_rec)
    res = run_bass_kernel_spmd(nc, in_maps, core_ids=list(range(N_CORES)))
    out = np.empty((N_ROWS, 2 * M), np.float32)
    for c in range(N_CORES):
        out[:, OUT_W * c:OUT_W * (c + 1)] = (
            res.results[c]["o"].astype(np.float32).T
        )
    return out


# revision 22
# speedup vs baseline: 1.0741x; 1.0003x over previous
"""Trainium2 Bass kernel for nn_BackwardTransformLayer (inverse wavelet step).

Math (polyphase form of the reference):
    g = flip(scaling_rec); g[1::2] *= -1
    out[i, 2u]   = sum_{j=0..3} g[2j]   * d[i, (u+j)   % M] + s[2j]   * a[i, (u+j)   % M]
    out[i, 2u+1] = sum_{j=0..3} g[2j+1] * d[i, (u+1+j) % M] + s[2j+1] * a[i, (u+1+j) % M]

Formulated as ONE banded matmul on the PE: for a block of BU=60 consecutive
u-columns, the 2*BU output columns are W.T @ X where
  - X [128, nrows]: partitions 0..63   = d^T rows [u0, u0+64)   (BU + 4 halo)
                    partitions 64..127 = a^T rows [u0, u0+64)
  - W [128, 2*BU] fp16: the shift-invariant banded coefficient matrix
    (4 taps per output column from each source), identical for every block.
PSUM accumulates in fp32; matmul cost is N cycles regardless of K, so this
does all 16 MACs per output element in 1/8 matmul-column-cycle each.

The kernel is purely HBM-bandwidth-bound, so both sides are 16-bit on the
wire: inputs fp16, and the fp32 PSUM result is rounded to fp16 during the
PSUM->SBUF drain and widened back to fp32 on the host (~5e-4 rel error,
gate is 2e-2).  The tail block (4 u-columns) gets a dedicated 16-partition
W slice (cols 2*BU..2*BU+2*TAIL_U) so its input tile is [16, nrows].

Sharding: by u-columns (embarrassingly parallel, 4-col halo replicated
host-side): core c handles u in [1024c, 1024(c+1)), all 4096 rows.  Inputs
are packed host-side into per-block [128, 4096] fp16 tiles (1 MB DMAs, all
128 partitions); output is computed transposed ([2048, 4096] per core) and
transposed back on the host.

Per core: 17 full blocks (60 u) + 1 tail block (4 u): 18 in-DMAs (sync
ring), 144 matmuls (N=512), 144 PSUM drains (ACT/DVE alternating, casting
to fp16), 18 out-DMAs (ACT ring).
"""

import numpy as np

P = 128                      # SBUF partitions
M = 8192                     # input columns (u dimension)
N_ROWS = 4096
N_CORES = 8
FLEN = 8
U_PER_CORE = M // N_CORES    # 1024
BU = 60                      # u-columns per full block (BU + 4 halo = 64 per source)
NBLK_FULL = 17               # 17 * 60 = 1020
TAIL_U = U_PER_CORE - NBLK_FULL * BU   # 4
TAIL_K = 2 * (TAIL_U + 4)    # 16 input partitions for the tail block
NBLK = NBLK_FULL + 1         # 18
OUT_W = 2 * U_PER_CORE       # 2048 output columns per core
NCHUNK = N_ROWS // 512       # 8 moving-dim chunks of 512 rows
X_ROWS = NBLK_FULL * P + TAIL_K        # 2192
WCOLS = 2 * BU + 2 * TAIL_U  # 128: full-block W | tail W

_CACHE = {}


def _build(reps=1, staggered=False, in_rings=("sync",)):
    import contextlib

    import concourse.bacc as bacc
    import concourse.mybir as mybir
    from concourse.tile import TileContext

    f32 = mybir.dt.float32
    f16 = mybir.dt.float16

    nc = bacc.Bacc("TRN2", target_bir_lowering=False, debug=False)
    x = nc.dram_tensor("x", [X_ROWS, N_ROWS], f16, kind="ExternalInput")
    w = nc.dram_tensor("w", [P, WCOLS], f16, kind="ExternalInput")
    o = nc.dram_tensor("o", [OUT_W, N_ROWS], f16, kind="ExternalOutput")

    with TileContext(nc) as tc:
        with (
            tc.tile_pool(name="const", bufs=1) as const_pool,
            tc.tile_pool(name="xin", bufs=NBLK) as xin_pool,
            tc.tile_pool(name="out", bufs=3) as out_pool,
            tc.tile_pool(name="psum", bufs=8, space="PSUM") as psum_pool,
        ):
            w_sb = const_pool.tile([P, WCOLS], f16)
            nc.sync.dma_start(out=w_sb[:], in_=w[:])

            rep_ctx = (
                tc.For_i(0, reps, 1, staggered_reset=staggered)
                if reps > 1
                else contextlib.nullcontext()
            )
            with rep_ctx:
                order = [NBLK - 1] + list(range(NBLK - 1))
                tiles = {}
                for b in order:
                    x_t = xin_pool.tile([P, N_ROWS], f16, tag="x")
                    in_eng = getattr(nc, in_rings[b % len(in_rings)])
                    if b < NBLK_FULL:
                        in_eng.dma_start(
                            out=x_t[:], in_=x[b * P:(b + 1) * P, :]
                        )
                    else:
                        in_eng.dma_start(
                            out=x_t[:TAIL_K, :],
                            in_=x[b * P:b * P + TAIL_K, :],
                        )
                    tiles[b] = x_t
                for b in order:
                    x_t = tiles[b]
                    if b < NBLK_FULL:
                        bw = 2 * BU
                        w_ap = w_sb[:, :bw]
                    else:
                        bw = 2 * TAIL_U
                        w_ap = w_sb[:TAIL_K, 2 * BU:2 * BU + bw]
                    o_t = out_pool.tile([2 * BU, N_ROWS], f16, tag="o")
                    r0 = 2 * BU * b
                    for ch in range(NCHUNK):
                        n0 = 512 * ch
                        ps = psum_pool.tile([2 * BU, 512], f32, tag="ps")
                        nc.tensor.matmul(
                            ps[:bw, :],
                            w_ap,
                            x_t[:w_ap.partition_size(), n0:n0 + 512],
                            start=True,
                            stop=True,
                        )
                        if ch % 2 == 0:
                            nc.scalar.copy(out=o_t[:bw, n0:n0 + 512], in_=ps[:bw, :])
                        else:
                            nc.vector.tensor_copy(
                                out=o_t[:bw, n0:n0 + 512], in_=ps[:bw, :]
                            )
                    nc.scalar.dma_start(out=o[r0:r0 + bw, :], in_=o_t[:bw, :])
    nc.compile()
    return nc


def _make_w(scaling, scaling_rec):
    s = np.asarray(scaling, dtype=np.float64)
    sr = np.asarray(scaling_rec, dtype=np.float64)
    g = sr[::-1].copy()
    g[1::2] *= -1.0

    w = np.zeros((P, WCOLS), np.float64)
    for ul in range(BU):
        for j in range(4):
            w[ul + j, 2 * ul] = g[2 * j]
            w[ul + 1 + j, 2 * ul + 1] = g[2 * j + 1]
            w[64 + ul + j, 2 * ul] = s[2 * j]
            w[64 + ul + 1 + j, 2 * ul + 1] = s[2 * j + 1]
    # tail block: d rows on partitions 0..TAIL_U+3, a rows on the next 8
    half = TAIL_U + 4
    for ul in range(TAIL_U):
        for j in range(4):
            w[ul + j, 2 * BU + 2 * ul] = g[2 * j]
            w[ul + 1 + j, 2 * BU + 2 * ul + 1] = g[2 * j + 1]
            w[half + ul + j, 2 * BU + 2 * ul] = s[2 * j]
            w[half + ul + 1 + j, 2 * BU + 2 * ul + 1] = s[2 * j + 1]
    return w.astype(np.float16)


def make_in_maps(details, approximation, scaling, scaling_rec):
    d16 = np.asarray(details, dtype=np.float16)
    a16 = np.asarray(approximation, dtype=np.float16)
    # circular halo: block slices reach at most col 1024c + 1020 + 8 <= M + 8
    dT = np.ascontiguousarray(
        np.concatenate([d16, d16[:, :FLEN]], axis=1).T
    )  # [M+8, N_ROWS]
    aT = np.ascontiguousarray(np.concatenate([a16, a16[:, :FLEN]], axis=1).T)
    w_np = _make_w(scaling, scaling_rec)

    in_maps = []
    for c in range(N_CORES):
        xc = np.zeros((X_ROWS, N_ROWS), np.float16)
        for b in range(NBLK_FULL):
            u0 = U_PER_CORE * c + BU * b
            xc[b * P:b * P + 64] = dT[u0:u0 + 64]
            xc[b * P + 64:b * P + 128] = aT[u0:u0 + 64]
        u0t = U_PER_CORE * c + BU * NBLK_FULL
        bt = NBLK_FULL * P
        half = TAIL_U + 4
        xc[bt:bt + half] = dT[u0t:u0t + half]
        xc[bt + half:bt + 2 * half] = aT[u0t:u0t + half]
        in_maps.append({"x": xc, "w": w_np})
    return in_maps


def kernel(details, approximation, scaling, scaling_rec):
    if "nc" not in _CACHE:
        _CACHE["nc"] = _build()
    nc = _CACHE["nc"]

    from concourse.bass_utils import run_bass_kernel_spmd

    in_maps = make_in_maps(details, approximation, scaling, scaling_rec)
    res = run_bass_kernel_spmd(nc, in_maps, core_ids=list(range(N_CORES)))
    out = np.empty((N_ROWS, 2 * M), np.float32)
    for c in range(N_CORES):
        out[:, OUT_W * c:OUT_W * (c + 1)] = (
            res.results[c]["o"].astype(np.float32).T
        )
    return out
